# revision 67
# baseline (speedup 1.0000x reference)
"""DualPathTransformer Trainium2 kernel.

Sharding: 8 cores = batch(4) x query-half(2). Each core processes one batch
and 1024 query tokens; K/V work is duplicated within a batch pair. No
device collectives: partial pooled projections are summed on the host.

SPMD uniformity trick: each core receives its batch token-ROTATED so that
its query tokens sit at rotated positions [512, 1536). Global attention is
permutation-invariant over keys; the local band structure is encoded in
host-prepped per-core mask tiles in true original coordinates. The program
is identical on all cores; only input data differs.

Layouts: activations feature-major (hT = [feature partitions, tokens]) for
matmuls; token-major (tokens on partitions) for layernorm stages. Scores
are computed transposed (keys on partitions) so softmax denominators come
free from a ones-row appended to V, and the AV matmul needs no transposes.

Precision: residual stream and weights fp32/f32r; attention q/k/v/probs and
post-attention projections bf16 (error contribution ~1e-3 of the stream).
"""

import numpy as np
import ml_dtypes
from contextlib import ExitStack

import concourse.bass as bass
import concourse.bacc as bacc
import concourse.tile as tile
import concourse.mybir as mybir
from concourse.bass_utils import run_bass_kernel_spmd

F32R = mybir.dt.float32r
F32 = mybir.dt.float32
BF16 = mybir.dt.bfloat16
AF = mybir.ActivationFunctionType
ALU = mybir.AluOpType

B, S, DIN, D, H, DOUT, W = 4, 2048, 256, 512, 8, 128, 64
HD = D // H          # 64
DFF = 2 * D          # 1024
NQ = S // 2          # 1024 queries per core
N_CORES = 8
Q0 = 512             # rotated position of first query token (uniform)
KL0, KL1 = 384, 1664   # local K/V window in rotated coords (10 ptiles)
NKL = KL1 - KL0        # 1280
DELTAS = (-128, 0, 128, 256, 384, 512)   # local kblock offsets rel. to qblock
# stripe (bounding qq range) per delta, qblock-relative
STRIPE = {-128: (0, 32), 0: (0, 160), 128: (96, 288),
          256: (224, 416), 384: (352, 512), 512: (480, 512)}
EDGE_DELTAS = (-128, 512)          # AV mms sliced to the stripe
SCALE = 1.0 / float(np.sqrt(HD))
EPS = 1e-5

_CACHE = {}
GLOBAL_KV_ON_ACT = False
LOCAL_KV_ON_ACT = True


def _build(flags, debug=False):
    (use_bqkv_l, use_bqkv_g, use_bo, use_gate_b, use_b1, use_b2,
     use_n1g, use_n1b, use_n2g, use_n2b, use_n3g) = flags

    nc = bacc.Bacc("TRN2", target_bir_lowering=False, debug=False)

    def din(name, shape, dt=F32R):
        return nc.dram_tensor(name, list(shape), dt, kind="ExternalInput").ap()

    xT = din("xT", [DIN, S], BF16)
    posb = din("posb", [D, S], BF16)
    win = din("win", [DIN, D], BF16)
    wqkv_l = din("wqkv_l", [3, D, D], BF16)
    wqkv_g = din("wqkv_g", [3, D, D], BF16)
    wo2 = din("wo2", [2, D, D], BF16)    # [0]=local, [1]=global
    gate_w = din("gate_w", [2 * D, D], BF16)
    w1 = din("w1", [D, DFF], BF16)
    w2 = din("w2", [DFF, D], BF16)
    outw = din("outw", [D, DOUT])
    masks_m = din("masks_m", [128, 4, 512], BF16)   # [kk, di, qq]
    masks_e = din("masks_e", [128, 2, 2, 32], BF16)  # [kk, de, qb, qq32]
    eye = din("eye", [128, 128], F32)
    poolw = din("poolw", [128, 1])
    if use_bqkv_l:
        bqkv_l = din("bqkv_l", [128, 3, 4], F32)
        bv_l = din("bv_l", [128, D], F32)
    if use_bqkv_g:
        bqkv_g = din("bqkv_g", [128, 3, 4], F32)
        bv_g = din("bv_g", [128, D], F32)
    if use_bo:
        bo2 = din("bo2", [128, 2, 4], F32)
    if use_gate_b:
        gate_b = din("gate_b", [128, 4], F32)
    if use_b1:
        b1 = din("b1", [128, 8], F32)
    if use_b2:
        b2b = din("b2b", [128, D], F32)
    if use_n1g:
        n1gb = din("n1gb", [128, D], F32)
    if use_n1b:
        n1bb = din("n1bb", [128, D], F32)
    if use_n2g:
        n2gb = din("n2gb", [128, D], F32)
    if use_n2b:
        n2bb = din("n2bb", [128, D], F32)
    if use_n3g:
        n3gb = din("n3gb", [128, D], F32)
    # n3_b handled on host (pooled mean is linear in it)

    po = nc.dram_tensor("po", [1, DOUT], F32, kind="ExternalOutput").ap()

    dbg = {}
    if debug:
        for nm, shp, dt_ in [("d_hT", [128, S], BF16), ("d_oTl", [128, NQ], BF16),
                             ("d_oTg", [128, NQ], BF16), ("d_gateT", [128, 512], BF16),
                             ("d_fusedT", [128, NQ], BF16), ("d_y1", [128, D], F32),
                             ("d_y3", [128, D], F32), ("d_pooled", [1, D], F32)]:
            dbg[nm] = nc.dram_tensor(nm, shp, dt_, kind="ExternalOutput").ap()

    f32 = lambda ap: ap.bitcast(F32)

    with tile.TileContext(nc) as tc, ExitStack() as top:
        # ---- psum pools (8 banks): ps = 4 x 1-bank slots (projection accs,
        # transposes, AV accumulators), ps2 = 2 x 2-bank slots (score tiles,
        # plus the long-lived pool/final accumulators which are tiny) ----
        ps = top.enter_context(tc.tile_pool(name="ps", bufs=4, space="PSUM"))
        ps2 = top.enter_context(tc.tile_pool(name="ps2", bufs=2, space="PSUM"))

        # ---- persistent pools (static tags, round-robin slot reuse) ----
        pers = top.enter_context(tc.tile_pool(name="pers", bufs=1))
        lnp = top.enter_context(tc.tile_pool(name="lnp", bufs=2))
        wp = top.enter_context(tc.tile_pool(name="wp", bufs=1))
        s4 = top.enter_context(tc.tile_pool(name="s4", bufs=1))     # [128,1024] bf16 tags
        s2 = top.enter_context(tc.tile_pool(name="s2", bufs=10))    # [128,512] f32
        qTp = top.enter_context(tc.tile_pool(name="qTp", bufs=4))   # [128,1024] bf16
        kTp = top.enter_context(tc.tile_pool(name="kTp", bufs=4))   # [128,2048] bf16
        hTp = top.enter_context(tc.tile_pool(name="hTp", bufs=1))
        Vp = top.enter_context(tc.tile_pool(name="Vp", bufs=16))    # [128,8,65] bf16
        ptgp = top.enter_context(tc.tile_pool(name="ptgp", bufs=3)) # pair bf16

        eye_sb = pers.tile([128, 128], F32, name="eye_sb")
        nc.sync.dma_start(eye_sb[:], eye[:])
        eyeb_sb = pers.tile([128, 128], BF16, name="eyeb_sb")
        nc.vector.tensor_copy(eyeb_sb[:], eye_sb[:])
        eps_sb = pers.tile([128, 1], F32, name="eps_sb")
        nc.vector.memset(eps_sb[:], EPS)
        # activation-table preload (Exp/Tanh/Sqrt) during the initial DMA wait
        dummy_sb = pers.tile([1, 1], F32, name="dummy_sb")
        for fn_ in (AF.Exp, AF.Tanh, AF.Sqrt):
            nc.scalar.activation(dummy_sb[:], eps_sb[0:1, :], fn_)
        # PE warmup: keep the tensor engine's dispatch queue busy during the
        # initial DMA wait so the real matmuls are priced at full p-state
        wsrc = pers.tile([128, 96], BF16, name="wsrc")
        nc.vector.memset(wsrc[:], 0.0)
        for i in range(64):
            wacc = ps.tile([96, 96], F32, name=f"wacc{i}", tag="ps")
            nc.tensor.matmul(wacc[:], wsrc[:, 0:96], wsrc[:, 0:96],
                             start=True, stop=True)
        poolw_sb = pers.tile([128, 1], F32R, name="poolw_sb")
        nc.sync.dma_start(poolw_sb[:], poolw[:])
        eps2_sb = pers.tile([128, 1], F32, name="eps2_sb")
        nc.vector.memset(eps2_sb[:], EPS * EPS)
        ones1_sb = pers.tile([1, 1], F32, name="ones1_sb")
        nc.vector.memset(ones1_sb[:], 1.0)

        def load_bias(ap_dram, shape, name):
            t = pers.tile(shape, F32, name=name)
            nc.sync.dma_start(t[:], ap_dram[:])
            return t
        bqkv_l_sb = load_bias(bqkv_l, [128, 3, 4], "bqkv_l_sb") if use_bqkv_l else None
        bv_l_sb = load_bias(bv_l, [128, D], "bv_l_sb") if use_bqkv_l else None
        bqkv_g_sb = load_bias(bqkv_g, [128, 3, 4], "bqkv_g_sb") if use_bqkv_g else None
        bv_g_sb = load_bias(bv_g, [128, D], "bv_g_sb") if use_bqkv_g else None
        bo2_sb = load_bias(bo2, [128, 2, 4], "bo2_sb") if use_bo else None
        gate_b_sb = load_bias(gate_b, [128, 4], "gate_b_sb") if use_gate_b else None
        b1_sb = load_bias(b1, [128, 8], "b1_sb") if use_b1 else None
        b2b_sb = load_bias(b2b, [128, D], "b2b_sb") if use_b2 else None
        n1gb_sb = load_bias(n1gb, [128, D], "n1gb_sb") if use_n1g else None
        n1bb_sb = load_bias(n1bb, [128, D], "n1bb_sb") if use_n1b else None
        n2gb_sb = load_bias(n2gb, [128, D], "n2gb_sb") if use_n2g else None
        n2bb_sb = load_bias(n2bb, [128, D], "n2bb_sb") if use_n2b else None
        n3gb_sb = load_bias(n3gb, [128, D], "n3gb_sb") if use_n3g else None

        # long-lived stream tiles (bf16: attention consumes h at bf16 anyway)
        hT = [hTp.tile([128, S], BF16, name=f"hT{m}", tag="hT", bufs=4)
              for m in range(4)]
        h_sb = [s2.tile([128, D], F32R, name=f"h{t}", tag="s2") for t in range(8)]

        # ============ Phase A: hT + h ======================================
        # DMA priority order on the sync queue: win + xT first (feed the
        # matmuls), then bf16 posb staging (the adds): hT = x@win + pos.
        sA = top.enter_context(ExitStack())
        pA = sA.enter_context(tc.tile_pool(name="pA", bufs=2))
        win_sb = pA.tile([128, 2, D], BF16, name="win_sb", tag="win", bufs=1)
        nc.sync.dma_start(win_sb[:], win.rearrange("(t p) n -> p t n", p=128))
        xTcs = []
        xTc = pA.tile([128, 2, 1024], BF16, name="xTc0", tag="xTc")
        nc.sync.dma_start(
            xTc[:], xT.rearrange("(t p) n -> p t n", p=128)[:, :, 0:1024])
        xTcs.append(xTc)
        pos_m = []
        for m in range(4):
            pm = pA.tile([128, S], BF16, name=f"posm{m}", tag="posm", bufs=4)
            nc.sync.dma_start(
                pm[:], posb.rearrange("(t p) n -> p t n", p=128)[:, m, :])
            pos_m.append(pm)
        xTc = pA.tile([128, 2, 1024], BF16, name="xTc1", tag="xTc")
        nc.sync.dma_start(
            xTc[:], xT.rearrange("(t p) n -> p t n", p=128)[:, :, 1024:2048])
        xTcs.append(xTc)
        # pos folds into the psum accumulation as eyeb.T @ pos; hT is then a
        # plain ACT copy out of psum (keeps DVE free in phase A)
        for c in range(2):
            xTc = xTcs[c]
            for m in range(4):
                for hh in range(2):
                    acc = ps.tile([128, 512], F32, name=f"psA{m}{c}{hh}",
                                  tag="ps")
                    for kt in range(2):
                        nc.tensor.matmul(
                            acc[:], win_sb[:, kt, m * 128:(m + 1) * 128],
                            xTc[:, kt, hh * 512:(hh + 1) * 512],
                            start=(kt == 0), stop=False)
                    c0 = c * 1024 + hh * 512
                    nc.tensor.matmul(
                        acc[:], eyeb_sb[:],
                        pos_m[m][:, c0:c0 + 512], start=False, stop=True)
                    nc.scalar.copy(hT[m][:, c0:c0 + 512], acc[:])
        sA.close()
        if debug:
            nc.sync.dma_start(dbg["d_hT"][:], hT[0][:])

        # ============ helper: qkv projection ================================
        def project_qkv(wqkv_sb, bias_sb, bv_sb, q_tiles, kT_tiles, v_tiles,
                        kT_lo, kT_hi, v_pt_lo, pfx, kv_on_act=True):
            for m in range(4):
                for n in range(2):
                    acc = ps.tile([128, 512], F32, name=f"{pfx}q{m}{n}", tag="ps")
                    for kt in range(4):
                        nc.tensor.matmul(
                            acc[:], wqkv_sb[:, 0, kt, m * 128:(m + 1) * 128],
                            hT[kt][:, Q0 + n * 512: Q0 + (n + 1) * 512],
                            start=(kt == 0), stop=(kt == 3))
                    dst = q_tiles[m].bitcast(BF16)[:, n * 512:(n + 1) * 512]
                    if bias_sb is not None:
                        nc.vector.tensor_scalar(
                            dst, acc[:], bias_sb[:, 0, m:m + 1], None,
                            op0=ALU.add)
                    else:
                        nc.vector.tensor_copy(dst, acc[:])
            nk = kT_hi - kT_lo
            for m in range(4):
                for off in range(0, nk, 512):
                    w_ = min(512, nk - off)
                    acc = ps.tile([128, 512], F32, name=f"{pfx}k{m}{off}",
                                  tag="ps")
                    for kt in range(4):
                        nc.tensor.matmul(
                            acc[:, 0:w_], wqkv_sb[:, 1, kt, m * 128:(m + 1) * 128],
                            hT[kt][:, kT_lo + off: kT_lo + off + w_],
                            start=(kt == 0), stop=(kt == 3))
                    dst = kT_tiles[m].bitcast(BF16)[:, off:off + w_]
                    if bias_sb is not None:
                        if kv_on_act:
                            nc.scalar.activation(dst, acc[:, 0:w_], AF.Identity,
                                                 bias=bias_sb[:, 1, m:m + 1])
                        else:
                            nc.vector.tensor_scalar(
                                dst, acc[:, 0:w_], bias_sb[:, 1, m:m + 1], None,
                                op0=ALU.add)
                    elif kv_on_act:
                        nc.scalar.copy(dst, acc[:, 0:w_])
                    else:
                        nc.vector.tensor_copy(dst, acc[:, 0:w_])
            for i, vt in enumerate(v_tiles):
                pt = v_pt_lo + i
                acc = ps.tile([128, 512], F32, name=f"{pfx}v{pt}", tag="ps")
                for kt in range(4):
                    nc.tensor.matmul(
                        acc[:], hT[kt][:, pt * 128:(pt + 1) * 128],
                        wqkv_sb[:, 2, kt, :], start=(kt == 0), stop=(kt == 3))
                dst3 = vt.bitcast(BF16)[:, :, 0:64]
                src3 = acc[:].rearrange("p (h e) -> p h e", h=8)
                if bv_sb is not None:
                    nc.vector.tensor_tensor(
                        dst3, src3,
                        f32(bv_sb[:]).rearrange("p (h e) -> p h e", h=8),
                        op=ALU.add)
                elif kv_on_act:
                    nc.scalar.copy(dst3, src3)
                else:
                    nc.vector.tensor_copy(dst3, src3)
                nc.gpsimd.memset(vt.bitcast(BF16)[:, :, 64:65], 1.0)

        # ============ helper: softmax-normalize attention head ==============
        def normalize(ps_o, oT_tile, r0, c0, pfx):
            recip = lnp.tile([1, 512], F32, name=f"{pfx}r", tag="recip")
            nc.vector.reciprocal(recip[:], ps_o[64:65, :])
            rb = lnp.tile([64, 512], F32, name=f"{pfx}rb", tag="rb")
            nc.gpsimd.partition_broadcast(rb[:], recip[:])
            nc.vector.tensor_tensor(
                oT_tile.bitcast(BF16)[r0:r0 + 64, c0:c0 + 512],
                ps_o[0:64, :], rb[:], op=ALU.mult)


        # copy-first variant: one DVE copy releases the psum bank, the rest
        # of the chain runs from SBUF off the bank-reuse critical path
        def normalize2(ps_o, oT_tile, r0, c0, pfx):
            osb = lnp.tile([65, 512], F32, name=f"{pfx}o", tag="osb", bufs=2)
            nc.vector.tensor_copy(osb[:], ps_o[0:65, :])
            recip = lnp.tile([1, 512], F32, name=f"{pfx}r", tag="recip")
            nc.vector.reciprocal(recip[:], osb[64:65, :])
            rb = lnp.tile([64, 512], F32, name=f"{pfx}rb", tag="rb")
            nc.gpsimd.partition_broadcast(rb[:], recip[:])
            nc.vector.tensor_tensor(
                oT_tile.bitcast(BF16)[r0:r0 + 64, c0:c0 + 512],
                osb[0:64, :], rb[:], op=ALU.mult)

        # ============ helper: out-projection (feature-major) ================
        def out_proj_mn(oT, outT, li, m, n, pfx, copy_on_act=True):
            acc = ps.tile([128, 512], F32, name=f"{pfx}{m}{n}", tag="ps")
            for kt in range(4):
                nc.tensor.matmul(
                    acc[:], wo_sb[:, li, kt, m * 128:(m + 1) * 128],
                    oT[kt].bitcast(BF16)[:, n * 512:(n + 1) * 512],
                    start=(kt == 0), stop=(kt == 3))
            dst = outT[m].bitcast(BF16)[:, n * 512:(n + 1) * 512]
            if use_bo:
                nc.scalar.activation(dst, acc[:], AF.Identity,
                                     bias=bo2_sb[:, li, m:m + 1])
            elif copy_on_act:
                nc.scalar.copy(dst, acc[:])
            else:
                nc.vector.tensor_copy(dst, acc[:])

        def out_proj(oT, outT, wo_sb_, li, pfx):
            for m in range(4):
                for n in range(2):
                    out_proj_mn(oT, outT, li, m, n, pfx)

        # ============ Phase B: local qkv ====================================
        qT_l = [qTp.tile([128, NQ], BF16, name=f"qTl{m}", tag="qT")
                for m in range(4)]
        kT_l = [kTp.tile([128, S], BF16, name=f"kTl{m}", tag="kT")
                for m in range(4)]
        V_l = [Vp.tile([128, 8, 65], BF16, name=f"Vl{pt}", tag="V")
               for pt in range(KL0 // 128, KL1 // 128)]
        wqkv_l_sb = wp.tile([128, 3, 4, D], BF16, name="wqkv_l_sb", tag="wbig")
        nc.sync.dma_start(
            wqkv_l_sb[:], wqkv_l.rearrange("w (t p) d -> p w t d", p=128))
        wo_sb = wp.tile([128, 2, 4, D], BF16, name="wo_sb", tag="wo2nd")
        nc.sync.dma_start(wo_sb[:], wo2.rearrange("w (t p) d -> p w t d", p=128))
        project_qkv(wqkv_l_sb, bqkv_l_sb, bv_l_sb, qT_l, kT_l, V_l,
                    KL0, KL1, KL0 // 128, "Bl", kv_on_act=LOCAL_KV_ON_ACT)
        # token-major h for core's tokens (rotated [512, 1536)); emitted after
        # phase B so it doesn't gate the local qkv projections. One psum bank
        # collects all four feature blocks -> single copy per t.
        for t in range(8):
            ptr = ps.tile([128, 512], BF16, name=f"ptrA{t}", tag="ps")
            for m in range(4):
                nc.tensor.transpose(
                    ptr[:, m * 128:(m + 1) * 128],
                    hT[m][:, Q0 + t * 128: Q0 + (t + 1) * 128],
                    eyeb_sb[:])
            nc.vector.tensor_copy(h_sb[t][:], ptr[:])
        # wqkv_g reuses the wbig slot; emit its DMA now (scalar queue) so the
        # transfer runs during phase C, right after phase B's last wbig read
        wqkv_g_sb = wp.tile([128, 3, 4, D], BF16, name="wqkv_g_sb", tag="wbig")
        nc.scalar.dma_start(
            wqkv_g_sb[:], wqkv_g.rearrange("w (t p) d -> p w t d", p=128))

        # ============ Phase C: local (band) attention + out-proj ============
        oT_l = [s4.tile([128, NQ], BF16, name=f"oTl{m}", tag="s4a", bufs=4)
                for m in range(4)]
        with ExitStack() as sC:
            pC = sC.enter_context(tc.tile_pool(name="pC", bufs=1))
            masks_m_sb = pC.tile([128, 4, 512], BF16, name="masks_m_sb")
            nc.scalar.dma_start(masks_m_sb[:], masks_m[:])
            masks_e_sb = pC.tile([128, 2, 2, 32], BF16, name="masks_e_sb")
            nc.sync.dma_start(masks_e_sb[:], masks_e[:])
            MAIN_DELTAS = (0, 128, 256, 384)
            # stripe-sliced AV. start=True on the first matmul marks the whole
            # 2KB psum zero-region pending-zero, so every later slice sees a
            # clean 0 on first touch and plain accumulation is correct.
            AV_PLAN = [(0, 0, 160, True), (128, 160, 288, False),
                       (256, 288, 416, False), (384, 416, 512, False),
                       (-128, 0, 32, False), (128, 96, 160, False),
                       (256, 224, 288, False), (384, 352, 416, False),
                       (512, 480, 512, False)]

            def local_scores(qb, hp, PT, mid=None):
                q0 = Q0 + qb * 512
                for di, dd in enumerate(MAIN_DELTAS):
                    if di == 2 and mid is not None:
                        mid()
                    qq0, qq1 = STRIPE[dd]
                    rel = q0 + dd - KL0
                    sc2 = ps2.tile([128, 2, 512], F32,
                                   name=f"psC{qb}{hp}{di}", tag="ps2")
                    for ab in range(2):
                        r0 = ab * 64
                        nc.tensor.matmul(
                            sc2[:, ab, qq0:qq1],
                            kT_l[hp].bitcast(BF16)[r0:r0 + 64, rel:rel + 128],
                            qT_l[hp].bitcast(BF16)
                            [r0:r0 + 64, qb * 512 + qq0: qb * 512 + qq1],
                            start=True, stop=False, tile_position=(r0, 0))
                        nc.tensor.matmul(
                            sc2[:, ab, qq0:qq1], eyeb_sb[:],
                            masks_m_sb[:, di, qq0:qq1],
                            start=False, stop=True)
                    pt_t = PT[dd]
                    nc.scalar.activation(
                        pt_t[:, :, qq0:qq1], sc2[:, :, qq0:qq1],
                        AF.Exp, scale=SCALE)
                for de_i, de in enumerate(EDGE_DELTAS):
                    qq0, qq1 = STRIPE[de]
                    rel = q0 + de - KL0
                    sc2 = ps2.tile([128, 2, 512], F32,
                                   name=f"psCe{qb}{hp}{de_i}", tag="ps2")
                    for ab in range(2):
                        r0 = ab * 64
                        nc.tensor.matmul(
                            sc2[:, ab, 0:32],
                            kT_l[hp].bitcast(BF16)[r0:r0 + 64, rel:rel + 128],
                            qT_l[hp].bitcast(BF16)
                            [r0:r0 + 64, qb * 512 + qq0: qb * 512 + qq1],
                            start=True, stop=False, tile_position=(r0, 0))
                        nc.tensor.matmul(
                            sc2[:, ab, 0:32], eyeb_sb[:],
                            masks_e_sb[:, de_i, qb, :],
                            start=False, stop=True)
                    pt_t = PT[de]
                    nc.scalar.activation(
                        pt_t[:], sc2[:, :, 0:32], AF.Exp, scale=SCALE)

            def local_av(qb, hp, PT):
                q0 = Q0 + qb * 512
                for ab in range(2):
                    head = 2 * hp + ab
                    po_t = ps.tile([65, 512], F32, name=f"psoC{qb}{hp}{ab}",
                                   tag="ps")
                    for i, (dd, a0, a1, first) in enumerate(AV_PLAN):
                        pq0 = STRIPE[dd][0]
                        nc.tensor.matmul(
                            po_t[:, a0:a1],
                            V_l[(q0 + dd - KL0) // 128].bitcast(BF16)[:, head, :],
                            PT[dd][:, ab, a0 - pq0:a1 - pq0]
                            if dd in EDGE_DELTAS else PT[dd][:, ab, a0:a1],
                            start=first, stop=(i == len(AV_PLAN) - 1),
                            skip_group_check=True)
                    normalize(po_t, oT_l[hp], ab * 64, qb * 512,
                              f"nC{qb}{hp}{ab}")

            pendC = []
            for qb in range(2):
                for hp in range(4):
                    PT = {}
                    for di, dd in enumerate(MAIN_DELTAS):
                        PT[dd] = pC.tile([128, 2, 512], BF16,
                                         name=f"PTl{qb}{hp}{di}",
                                         tag=f"PTm{di}", bufs=3)
                    for de_i, de in enumerate(EDGE_DELTAS):
                        PT[de] = pC.tile([128, 2, 32], BF16,
                                         name=f"PTe{qb}{hp}{de_i}",
                                         tag=f"PTe{de_i}", bufs=3)
                    pc = pendC.pop(0) if len(pendC) >= 2 else None
                    local_scores(qb, hp, PT,
                                 mid=(lambda: local_av(*pc)) if pc else None)
                    pendC.append((qb, hp, PT))
            for pc in pendC:
                local_av(*pc)
        if debug:
            nc.sync.dma_start(dbg["d_oTl"][:], oT_l[0].bitcast(BF16)[:])

        localT = [s4.tile([128, NQ], BF16, name=f"localT{m}", tag="s4b", bufs=4)
                  for m in range(4)]

        # ============ Phase D: global qkv ===================================
        qT_g = [qTp.tile([128, NQ], BF16, name=f"qTg{m}", tag="qT")
                for m in range(4)]
        kT_g = [kTp.tile([128, S], BF16, name=f"kTg{m}", tag="kT")
                for m in range(4)]
        V_g = [Vp.tile([128, 8, 65], BF16, name=f"Vg{pt}", tag="V")
               for pt in range(16)]
        project_qkv(wqkv_g_sb, bqkv_g_sb, bv_g_sb, qT_g, kT_g, V_g, 0, S, 0, "Dg", kv_on_act=GLOBAL_KV_ON_ACT)
        # gate_w reuses the wbig slot after phase D's last read; emit now so
        # the transfer overlaps phase E
        gate_w_sb = wp.tile([128, 8, D], BF16, name="gate_w_sb", tag="wbig")
        nc.scalar.dma_start(gate_w_sb[:],
                          gate_w.rearrange("(t p) d -> p t d", p=128))

        # ============ Phase E: global attention, out-projs + gate woven =====
        # Phase E is Act-bound (exp); the PE idles ~190ns per k-block. The
        # local out-proj, the qb0 global out-proj, and the n=0 gate matmuls
        # are woven into that slack as filler. Gate matmul results are parked
        # as relu'd bf16 (gpark); tanh+fuse run after E.
        oT_g = [s4.tile([128, NQ], BF16, name=f"oTg{m}", tag="s4c", bufs=8)
                for m in range(4)]
        globalT = [s4.tile([128, NQ], BF16, name=f"globalT{m}", tag="s4c",
                           bufs=8) for m in range(4)]
        gpark = [s4.tile([128, NQ], BF16, name=f"gpark{m}", tag="gpk", bufs=4)
                 for m in range(4)]
        fusedT = [s4.tile([128, NQ], BF16, name=f"fusedT{m}", tag="s4a", bufs=4)
                  for m in range(4)]
        cat = localT + globalT

        def fuse_chain(m, n):
            # fused = global + tanh(relu(gate)) * (local - global)
            gt = lnp.tile([128, 512], BF16, name=f"gt{m}{n}", tag="gt", bufs=1)
            nc.scalar.activation(
                gt[:], gpark[m].bitcast(BF16)[:, n * 512:(n + 1) * 512],
                AF.Tanh)
            if debug and m == 0 and n == 0:
                nc.sync.dma_start(dbg["d_gateT"][:], gt[:])
            lsl = localT[m].bitcast(BF16)[:, n * 512:(n + 1) * 512]
            gsl = globalT[m].bitcast(BF16)[:, n * 512:(n + 1) * 512]
            tmp = lnp.tile([128, 512], BF16, name=f"tmpG{m}{n}", tag="tmpG",
                           bufs=1)
            nc.gpsimd.tensor_tensor(tmp[:], lsl, gsl, op=ALU.subtract)
            nc.vector.tensor_tensor(tmp[:], tmp[:], gt[:], op=ALU.mult)
            nc.vector.tensor_tensor(
                fusedT[m].bitcast(BF16)[:, n * 512:(n + 1) * 512],
                tmp[:], gsl, op=ALU.add)

        def gate_mm(m, n, copy_on_act=False):
            acc = ps.tile([128, 512], F32, name=f"psG{m}{n}", tag="ps")
            for kt in range(8):
                nc.tensor.matmul(
                    acc[:], gate_w_sb[:, kt, m * 128:(m + 1) * 128],
                    cat[kt].bitcast(BF16)[:, n * 512:(n + 1) * 512],
                    start=(kt == 0), stop=(kt == 7))
            dst = gpark[m].bitcast(BF16)[:, n * 512:(n + 1) * 512]
            if use_gate_b:
                nc.vector.tensor_scalar(
                    dst, acc[:], gate_b_sb[:, m:m + 1], 0.0,
                    op0=ALU.add, op1=ALU.max)
            elif copy_on_act:
                nc.scalar.activation(dst, acc[:], AF.Relu)
            else:
                nc.vector.tensor_scalar(dst, acc[:], 0.0, None, op0=ALU.max)

        # filler schedule: group index g = qb*4 + hp, woven at kt 5 and 11
        fillers = {
            0: [lambda: out_proj_mn(oT_l, localT, 0, 0, 0, "psFl", False),
                lambda: out_proj_mn(oT_l, localT, 0, 0, 1, "psFl", False)],
            1: [lambda: out_proj_mn(oT_l, localT, 0, 1, 0, "psFl", False),
                lambda: out_proj_mn(oT_l, localT, 0, 1, 1, "psFl", False)],
            2: [lambda: out_proj_mn(oT_l, localT, 0, 2, 0, "psFl", False),
                lambda: out_proj_mn(oT_l, localT, 0, 2, 1, "psFl", False)],
            3: [lambda: out_proj_mn(oT_l, localT, 0, 3, 0, "psFl", False),
                lambda: out_proj_mn(oT_l, localT, 0, 3, 1, "psFl", False)],
            4: [lambda: out_proj_mn(oT_g, globalT, 1, 0, 0, "psFg", False),
                lambda: out_proj_mn(oT_g, globalT, 1, 1, 0, "psFg", False)],
            5: [lambda: out_proj_mn(oT_g, globalT, 1, 2, 0, "psFg", False),
                lambda: out_proj_mn(oT_g, globalT, 1, 3, 0, "psFg", False)],
            6: [lambda: gate_mm(0, 0), lambda: gate_mm(1, 0)],
            7: [lambda: gate_mm(2, 0), lambda: gate_mm(3, 0)],
        }
        prevE = None
        for qb in range(2):
            for hp in range(4):
                g = qb * 4 + hp
                po_ts = [ps.tile([65, 512], F32, name=f"psoE{qb}{hp}{ab}",
                                 tag="ps") for ab in range(2)]
                for kt in range(16):
                    sc2 = ps2.tile([128, 2, 512], F32,
                                   name=f"psE{qb}{hp}{kt}", tag="ps2")
                    for ab in range(2):
                        r0 = ab * 64
                        nc.tensor.matmul(
                            sc2[:, ab, :], kT_g[hp].bitcast(BF16)
                            [r0:r0 + 64, kt * 128:(kt + 1) * 128],
                            qT_g[hp].bitcast(BF16)
                            [r0:r0 + 64, qb * 512:(qb + 1) * 512],
                            start=True, stop=True, tile_position=(r0, 0))
                    ptg = ptgp.tile([128, 2, 512], BF16,
                                    name=f"ptg{qb}{hp}{kt}", tag="ptg")
                    nc.scalar.activation(ptg[:], sc2[:], AF.Exp, scale=SCALE)
                    if kt == 5 or kt == 11:
                        fillers[g][0 if kt == 5 else 1]()
                    if prevE is not None:
                        pg, php, pqb, pkt, pptg, ppo = prevE
                        for ab in range(2):
                            nc.tensor.matmul(
                                ppo[ab][:],
                                V_g[pkt].bitcast(BF16)[:, 2 * php + ab, :],
                                pptg[:, ab, :], start=(pkt == 0),
                                stop=(pkt == 15), skip_group_check=True)
                        if pkt == 15:
                            for ab in range(2):
                                normalize2(ppo[ab], oT_g[php], ab * 64,
                                           pqb * 512, f"nE{pqb}{php}{ab}")
                    prevE = (g, hp, qb, kt, ptg, po_ts)
        # flush the last AV + normalize
        pg, php, pqb, pkt, pptg, ppo = prevE
        for ab in range(2):
            nc.tensor.matmul(
                ppo[ab][:], V_g[pkt].bitcast(BF16)[:, 2 * php + ab, :],
                pptg[:, ab, :], start=False, stop=True, skip_group_check=True)
        for ab in range(2):
            normalize2(ppo[ab], oT_g[php], ab * 64, pqb * 512,
                       f"nE{pqb}{php}{ab}")
        # n=0 fuse chains run on Act/Pool/DVE under the post-E PE work
        for m in range(4):
            fuse_chain(m, 0)
        if debug:
            nc.sync.dma_start(dbg["d_oTg"][:], oT_g[0].bitcast(BF16)[:])

        # ============ tail: LN1 chains overlap the n=1 out-proj/gate ========

        # ===== layernorm helper (token-major [128, D]) ======================
        def layernorm(dst, src_ap, g_sb, b_sb, pfx):
            stats = lnp.tile([128, 6], F32, name=f"{pfx}st", tag="lnst")
            nc.vector.bn_stats(stats[:], src_ap)
            mv = lnp.tile([128, 2], F32, name=f"{pfx}mv", tag="lnmv")
            nc.vector.bn_aggr(mv[:], stats[:])
            std = lnp.tile([128, 1], F32, name=f"{pfx}sd", tag="lnsd")
            nc.scalar.activation(std[:], mv[:, 1:2], AF.Sqrt, bias=eps_sb[:])
            rstd = lnp.tile([128, 1], F32, name=f"{pfx}rs", tag="lnrs")
            nc.vector.reciprocal(rstd[:], std[:])
            if g_sb is not None:
                tmp = lnp.tile([128, D], F32, name=f"{pfx}tmp", tag="lntmp")
                nc.vector.tensor_scalar(
                    tmp[:], src_ap, mv[:, 0:1], rstd[:],
                    op0=ALU.subtract, op1=ALU.mult)
                if b_sb is not None:
                    nc.vector.tensor_tensor(dst, tmp[:], g_sb[:], op=ALU.mult)
                    nc.vector.tensor_tensor(dst, dst, b_sb[:], op=ALU.add)
                else:
                    nc.vector.tensor_tensor(dst, tmp[:], g_sb[:], op=ALU.mult)
            else:
                nc.vector.tensor_scalar(
                    dst, src_ap, mv[:, 0:1], rstd[:],
                    op0=ALU.subtract, op1=ALU.mult)
                if b_sb is not None:
                    nc.vector.tensor_tensor(dst, dst, b_sb[:], op=ALU.add)

        # ============ Phase H+I: x1 = h + fused^T; y1 = LN1; y1T ============
        # LN1 moments via Act accum_out (keeps DVE short); per-t chain:
        # PE transposes -> DVE add -> ACT sums -> DVE normalize -> PE ptrI
        y1 = [s2.tile([128, D], F32R, name=f"y1_{t}", tag="s2") for t in range(8)]
        y1T = [s4.tile([128, NQ], BF16, name=f"y1T{m}", tag="s4b", bufs=4)
               for m in range(4)]

        def ln1_t(t, x1):
            if use_n1g or use_n1b:
                layernorm(y1[t][:], x1[:], n1gb_sb, n1bb_sb, f"ln1_{t}")
            else:
                pfx = f"ln1_{t}"
                xs = lnp.tile([128, 2], F32, name=f"{pfx}xs", tag="lnxs")
                nc.scalar.activation(f32(y1[t][:]), x1[:], AF.Identity,
                                     accum_out=xs[:, 0:1])
                scr2 = lnp.tile([128, D], F32, name=f"{pfx}s2", tag="lnscr",
                                bufs=1)
                nc.scalar.activation(scr2[:], x1[:], AF.Square,
                                     accum_out=xs[:, 1:2])
                mv = lnp.tile([128, 2], F32, name=f"{pfx}mv", tag="lnmv")
                nc.vector.tensor_scalar(mv[:], xs[:], 1.0 / D, None,
                                        op0=ALU.mult)
                m2 = lnp.tile([128, 1], F32, name=f"{pfx}m2", tag="lnm2")
                nc.vector.tensor_tensor(m2[:], mv[:, 0:1], mv[:, 0:1],
                                        op=ALU.mult)
                var = lnp.tile([128, 1], F32, name=f"{pfx}vr", tag="lnvr")
                nc.vector.tensor_tensor(var[:], mv[:, 1:2], m2[:],
                                        op=ALU.subtract)
                std = lnp.tile([128, 1], F32, name=f"{pfx}sd", tag="lnsd")
                nc.scalar.activation(std[:], var[:], AF.Sqrt, bias=eps_sb[:])
                rstd = lnp.tile([128, 1], F32, name=f"{pfx}rs", tag="lnrs")
                nc.vector.reciprocal(rstd[:], std[:])
                nc.vector.tensor_scalar(
                    y1[t][:], x1[:], mv[:, 0:1], rstd[:],
                    op0=ALU.subtract, op1=ALU.mult)

        def ptrI_t(t):
            pt2 = ps.tile([128, 512], F32, name=f"ptrI{t}", tag="ps")
            for m in range(4):
                nc.tensor.transpose(
                    pt2[:, m * 128:(m + 1) * 128],
                    f32(y1[t][:, m * 128:(m + 1) * 128]), eye_sb[:])
                nc.scalar.copy(
                    y1T[m].bitcast(BF16)[:, t * 128:(t + 1) * 128],
                    pt2[:, m * 128:(m + 1) * 128])

        # ============ Phase J: FFN + LN2 + LN3; Phase K: pool + out =========
        z1T = [s4.tile([128, NQ], BF16, name=f"z1T{m}", tag="s4c", bufs=8)
               for m in range(8)]

        def ffn1_half(n):
            for m in range(8):
                acc = ps.tile([128, 512], F32, name=f"psJ1{m}{n}", tag="ps")
                for kt in range(4):
                    nc.tensor.matmul(
                        acc[:], w1_sb[:, kt, m * 128:(m + 1) * 128],
                        y1T[kt].bitcast(BF16)[:, n * 512:(n + 1) * 512],
                        start=(kt == 0), stop=(kt == 3))
                dst = z1T[m].bitcast(BF16)[:, n * 512:(n + 1) * 512]
                if use_b1:
                    nc.vector.tensor_scalar(
                        dst, acc[:], b1_sb[:, m:m + 1], 0.0,
                        op0=ALU.add, op1=ALU.max)
                else:
                    nc.scalar.activation(dst, acc[:], AF.Relu)

        def ptrH_t(t):
            ptr = ps.tile([128, 512], BF16, name=f"ptrH{t}", tag="ps")
            for m in range(4):
                nc.tensor.transpose(
                    ptr[:, m * 128:(m + 1) * 128],
                    fusedT[m].bitcast(BF16)[:, t * 128:(t + 1) * 128],
                    eyeb_sb[:])
            x1 = lnp.tile([128, D], F32, name=f"x1_{t}", tag="x1")
            nc.vector.tensor_tensor(x1[:], f32(h_sb[t][:]), ptr[:], op=ALU.add)
            ln1_t(t, x1)

        # t0-3 LN chains (DVE/ACT) overlap the n=1 out-proj + gate matmuls
        for t in range(4):
            ptrH_t(t)
        for m in range(4):
            out_proj_mn(oT_g, globalT, 1, m, 1, "psFg")
        # w2 reuses the wo2nd slot freed by out_proj_g
        w2_sb = wp.tile([128, 8, D], BF16, name="w2_sb", tag="wo2nd")
        nc.scalar.dma_start(w2_sb[:], w2.rearrange("(t p) d -> p t d", p=128))
        for m in range(4):
            gate_mm(m, 1, copy_on_act=False)
        # w1 reuses the wbig slot; all gate_w reads are done
        w1_sb = wp.tile([128, 4, DFF], BF16, name="w1_sb", tag="wbig")
        nc.scalar.dma_start(w1_sb[:], w1.rearrange("(t p) d -> p t d", p=128))
        for m in range(4):
            fuse_chain(m, 1)
        if debug:
            nc.sync.dma_start(dbg["d_fusedT"][:], fusedT[0].bitcast(BF16)[:])
        for t in range(4):
            ptrI_t(t)
        ffn1_half(0)
        for t in range(4, 8):
            ptrH_t(t)

        y3 = [s2.tile([128, D], F32R, name=f"y3_{t}", tag="s2") for t in range(8)]
        accp = ps2.tile([1, 512], F32, name="pspool", tag="ps2")

        def j2_t(t):
            acc = ps.tile([128, 512], F32, name=f"psJ2{t}", tag="ps")
            for kt in range(8):
                nc.tensor.matmul(
                    acc[:], z1T[kt].bitcast(BF16)[:, t * 128:(t + 1) * 128],
                    w2_sb[:, kt, :], start=(kt == 0), stop=(kt == 7))
            x2 = lnp.tile([128, D], F32, name=f"x2_{t}", tag="x2")
            nc.vector.tensor_tensor(x2[:], acc[:], f32(y1[t][:]), op=ALU.add)
            if use_b2:
                nc.vector.tensor_tensor(x2[:], x2[:], b2b_sb[:], op=ALU.add)
            if not (use_n2g or use_n2b or use_n3g):
                # LN3(LN2(x)) with unit gamma / zero beta collapses to one LN:
                # mean(LN2 out) == 0 exactly, var(LN2 out) = v/(v+eps), so
                # y3 = (x - m) / sqrt(v*(1+eps) + eps^2)
                pfx = f"ln23_{t}"
                stats = lnp.tile([128, 6], F32, name=f"{pfx}st", tag="lnst")
                nc.vector.bn_stats(stats[:], x2[:])
                mv = lnp.tile([128, 2], F32, name=f"{pfx}mv", tag="lnmv")
                nc.vector.bn_aggr(mv[:], stats[:])
                std = lnp.tile([128, 1], F32, name=f"{pfx}sd", tag="lnsd")
                nc.scalar.activation(std[:], mv[:, 1:2], AF.Sqrt,
                                     bias=eps2_sb[:], scale=1.0 + EPS)
                rstd = lnp.tile([128, 1], F32, name=f"{pfx}rs", tag="lnrs")
                nc.vector.reciprocal(rstd[:], std[:])
                nc.vector.tensor_scalar(
                    y3[t][:], x2[:], mv[:, 0:1], rstd[:],
                    op0=ALU.subtract, op1=ALU.mult)
            else:
                y2 = lnp.tile([128, D], F32, name=f"y2_{t}", tag="y2")
                layernorm(y2[:], x2[:], n2gb_sb, n2bb_sb, f"ln2_{t}")
                layernorm(y3[t][:], y2[:], n3gb_sb, None, f"ln3_{t}")
            nc.tensor.matmul(accp[:], poolw_sb[:], y3[t][:],
                             start=(t == 0), stop=(t == 7),
                             skip_group_check=True)

        # J2 for t0/t1 fills the PE while the t4-7 LN chains drain
        j2_t(0)
        j2_t(1)
        for t in range(4, 8):
            ptrI_t(t)
        ffn1_half(1)
        if debug:
            nc.sync.dma_start(dbg["d_y1"][:], f32(y1[0][:]))
        for t in range(2, 8):
            j2_t(t)
        if debug:
            nc.sync.dma_start(dbg["d_y3"][:], f32(y3[0][:]))

        outw_sb = lnp.tile([128, 4, DOUT], F32R, name="outw_sb", tag="x2",
                           bufs=2)
        nc.sync.dma_start(outw_sb[:], outw.rearrange("(t p) n -> p t n", p=128))
        pooled_sb = pers.tile([1, D], F32R, name="pooled_sb")
        nc.vector.tensor_copy(pooled_sb[:], accp[:])
        if debug:
            nc.sync.dma_start(dbg["d_pooled"][:], f32(pooled_sb[:]))
        # transpose pooled [1, 512] -> [128, 4] on-chip: out col m is
        # pooled[0, m*128:(m+1)*128].T @ [[1]]
        pooledT_ps = ps.tile([128, 4], F32, name="pooledT_ps", tag="ps")
        for m in range(4):
            nc.tensor.matmul(pooledT_ps[:, m:m + 1],
                             f32(pooled_sb)[0:1, m * 128:(m + 1) * 128],
                             ones1_sb[0:1, 0:1], start=True, stop=True,
                             skip_group_check=True)
        pooledT = pers.tile([128, 4], F32R, name="pooledT")
        nc.vector.tensor_copy(pooledT[:], pooledT_ps[:])
        accf = ps2.tile([1, 128], F32, name="psfin", tag="ps2")
        for kt in range(4):
            nc.tensor.matmul(accf[:], pooledT[:, kt:kt + 1], outw_sb[:, kt, :],
                             start=(kt == 0), stop=(kt == 3))
        po_sb = pers.tile([1, DOUT], F32, name="po_sb")
        nc.vector.tensor_copy(po_sb[:], accf[:])
        nc.sync.dma_start(po[:], po_sb[:])

    nc.compile()
    return nc


def _prep_inputs(inputs):
    """Host-side prep: returns (flags, in_maps for 8 cores, host_const)."""
    g = {k: np.asarray(v, dtype=np.float32) for k, v in inputs.items()}
    x, pos = g["x"], g["pos"]
    win_w, win_b = g["win_w"], g["win_b"]

    flags = (
        bool(np.any(g["l_bqkv"] != 0)), bool(np.any(g["g_bqkv"] != 0)),
        bool(np.any(g["l_bo"] != 0) or np.any(g["g_bo"] != 0)),
        bool(np.any(g["gate_b"] != 0)), bool(np.any(g["ffn_b1"] != 0)),
        bool(np.any(g["ffn_b2"] != 0)),
        bool(np.any(g["n1_g"] != 1)), bool(np.any(g["n1_b"] != 0)),
        bool(np.any(g["n2_g"] != 1)), bool(np.any(g["n2_b"] != 0)),
        bool(np.any(g["n3_g"] != 1)),
    )
    (use_bqkv_l, use_bqkv_g, use_bo, use_gate_b, use_b1, use_b2,
     use_n1g, use_n1b, use_n2g, use_n2b, use_n3g) = flags

    posT = pos[0].T + win_b[:, None]                      # [D, S]
    common = {
        "win": np.ascontiguousarray(win_w).astype(ml_dtypes.bfloat16),
        "wqkv_l": np.ascontiguousarray(g["l_wqkv"]).astype(ml_dtypes.bfloat16),
        "wqkv_g": np.ascontiguousarray(g["g_wqkv"]).astype(ml_dtypes.bfloat16),
        "wo2": np.stack([g["l_wo"], g["g_wo"]]).astype(ml_dtypes.bfloat16),
        "gate_w": g["gate_w"].astype(ml_dtypes.bfloat16),
        "w1": g["ffn_w1"].astype(ml_dtypes.bfloat16),
        "w2": g["ffn_w2"].astype(ml_dtypes.bfloat16),
        "outw": np.ascontiguousarray(g["out_w"]),
        "eye": np.eye(128, dtype=np.float32),
        "poolw": np.full((128, 1), 1.0 / S, dtype=np.float32),
    }
    perm = lambda b: b.reshape(-1, 4, 128).transpose(2, 0, 1).copy()
    if use_bqkv_l:
        common["bqkv_l"] = perm(g["l_bqkv"])
        common["bv_l"] = np.tile(g["l_bqkv"][2], (128, 1))
    if use_bqkv_g:
        common["bqkv_g"] = perm(g["g_bqkv"])
        common["bv_g"] = np.tile(g["g_bqkv"][2], (128, 1))
    if use_bo:
        common["bo2"] = perm(np.stack([g["l_bo"], g["g_bo"]]))
    if use_gate_b:
        common["gate_b"] = g["gate_b"].reshape(4, 128).T.copy()
    if use_b1:
        common["b1"] = g["ffn_b1"].reshape(8, 128).T.copy()
    if use_b2:
        common["b2b"] = np.tile(g["ffn_b2"], (128, 1))
    if use_n1g:
        common["n1gb"] = np.tile(g["n1_g"], (128, 1))
    if use_n1b:
        common["n1bb"] = np.tile(g["n1_b"], (128, 1))
    if use_n2g:
        common["n2gb"] = np.tile(g["n2_g"], (128, 1))
    if use_n2b:
        common["n2bb"] = np.tile(g["n2_b"], (128, 1))
    if use_n3g:
        common["n3gb"] = np.tile(g["n3_g"], (128, 1))

    # universal interior band masks (pure Toeplitz, no seam crossing)
    kk = np.arange(128)
    qq = np.arange(512)
    mk_m = np.zeros((128, 4, 512), dtype=np.float32)
    for di, d in enumerate((0, 128, 256, 384)):
        mk_m[:, di, :] = (np.abs(kk[:, None] + d - qq[None, :]) <= W // 2)
    mk_m = ((mk_m - 1.0) * 1e4).astype(ml_dtypes.bfloat16)

    hf_data = []
    for hf in range(2):
        q0c = NQ * hf
        shift = Q0 - q0c
        posb_rot = np.ascontiguousarray(
            np.roll(posT, shift, axis=1)).astype(ml_dtypes.bfloat16)
        mk_e = np.zeros((128, 2, 2, 32), dtype=np.float32)
        for qb in range(2):
            q0 = Q0 + qb * 512
            for de_i, d in enumerate(EDGE_DELTAS):
                qq0, qq1 = STRIPE[d]
                k_rot = q0 + d + kk[:, None]
                q_rot = q0 + np.arange(qq0, qq1)[None, :]
                orig_k = (k_rot - shift) % S
                orig_q = (q_rot - shift) % S
                mk_e[:, de_i, qb, :] = (np.abs(orig_k - orig_q) <= W // 2)
        mk_e = (mk_e - 1.0) * 1e4
        hf_data.append((posb_rot, mk_e.astype(ml_dtypes.bfloat16)))

    in_maps = []
    for core in range(N_CORES):
        b, hf = core // 2, core % 2
        shift = Q0 - NQ * hf
        posb_rot, mk_e = hf_data[hf]
        m = dict(common)
        m["xT"] = np.ascontiguousarray(np.roll(x[b].T, shift, axis=1)).astype(ml_dtypes.bfloat16)
        m["posb"] = posb_rot
        m["masks_m"] = mk_m
        m["masks_e"] = mk_e
        in_maps.append(m)

    host_const = g["n3_b"] @ g["out_w"] + g["out_b"]
    return flags, in_maps, host_const


def kernel(**inputs):
    flags, in_maps, host_const = _prep_inputs(inputs)
    if flags not in _CACHE:
        _CACHE[flags] = _build(flags)
    nc = _CACHE[flags]
    res = run_bass_kernel_spmd(nc, in_maps, core_ids=list(range(N_CORES)))
    out = np.zeros((B, DOUT), dtype=np.float32)
    for b in range(B):
        out[b] = (res.results[2 * b]["po"][0] + res.results[2 * b + 1]["po"][0]
                  + host_const)
    return out

